# revision 10
# baseline (speedup 1.0000x reference)
"""Entmax-1.5 attention kernel for Trainium2 (8 NeuronCores, head-sharded).

Computes, for inputs q,k,v [1,16,2048,64]:
  scores  = q @ k^T / sqrt(64)
  teacher = softmax(scores)
  student = entmax_{1.5}(scores)      (matches 50-iter bisection reference)
  out     = student @ v

Key algorithmic idea: for alpha=1.5 the entmax exponent is exactly 2, so
  p_i = max(y_i - tau, 0)^2 / s,   s(tau) = sum_i max(y_i - tau, 0)^2
with y = 0.5*scores/sqrt(D). s(tau) is convex, piecewise-quadratic and
decreasing, so tau* (where s=1) is found with a few Newton iterations from a
per-row analytic anchor tau0 = mean + T*sigma_hat (sigma_hat linear in the row
variance). Newton from below never overshoots; anchor overshoot (a handful of
rows) recovers because iterates are always re-derived from the full y tile.
Each iteration is one DVE tensor_scalar pass (u = relu(y - tau), fused row-sum
S1) plus one Square/stt pass (s = sum u^2 fused accumulation).
"""

import os
import numpy as np
from contextlib import ExitStack

import concourse.bass as bass
import concourse.mybir as mybir
import concourse.tile as tile
from concourse import bacc, masks
from concourse.bass_utils import run_bass_kernel_spmd

AF = mybir.ActivationFunctionType
ALU = mybir.AluOpType
F32 = mybir.dt.float32
ts = bass.ts

# ---- problem constants (hardcoded; kernel must be self-contained) ----
B, H, S, D = 1, 16, 2048, 64
NCORES = 8
HPC = H // NCORES          # heads per core = 2
NB = S // 128              # q blocks per head = 16
GROUP = 8                  # q blocks per convoy group
NM = 7                     # measure passes per row (incl. final)
HALF_C = 0.5 / (D ** 0.5)  # y = HALF_C * raw_scores
# sigma_hat = SIG_A * var + SIG_B (linear fit of sqrt over observed var range)
SIG_A = np.float32(0.8076871)
SIG_B = np.float32(0.2919374)
T_INIT = np.float32(1.75)

_CACHE = {}


def _build_program(n_blocks=NB, n_meas=NM):
    nc = bacc.Bacc(
        "TRN2", target_bir_lowering=False, debug=False, num_devices=NCORES
    )
    q_d = nc.dram_tensor("q", [HPC, S, D], F32, kind="ExternalInput").ap()
    k_d = nc.dram_tensor("k", [HPC, S, D], F32, kind="ExternalInput").ap()
    v_d = nc.dram_tensor("v", [HPC, S, D], F32, kind="ExternalInput").ap()
    o_d = nc.dram_tensor("o", [HPC, S, D], F32, kind="ExternalOutput").ap()
    st_d = nc.dram_tensor("st", [HPC, S, S], F32, kind="ExternalOutput").ap()
    te_d = nc.dram_tensor("te", [HPC, S, S], F32, kind="ExternalOutput").ap()

    with tile.TileContext(nc) as tc, ExitStack() as ctx:
        ident_pool = ctx.enter_context(tc.tile_pool(name="ident", bufs=1))
        ident = ident_pool.tile([128, 128], F32)
        masks.make_identity(nc, ident[:])

        stage = ctx.enter_context(tc.tile_pool(name="stage", bufs=2))
        vpool = ctx.enter_context(tc.tile_pool(name="vpool", bufs=2))
        qtp = ctx.enter_context(tc.tile_pool(name="qt", bufs=1))
        ktp = ctx.enter_context(tc.tile_pool(name="kt", bufs=1))
        tpool = ctx.enter_context(tc.tile_pool(name="texp", bufs=2))
        tnpool = ctx.enter_context(tc.tile_pool(name="tn", bufs=2))
        upool = ctx.enter_context(tc.tile_pool(name="u", bufs=GROUP + 1))
        wpool = ctx.enter_context(tc.tile_pool(name="w", bufs=1))
        ppool = ctx.enter_context(tc.tile_pool(name="p", bufs=2))
        pnpool = ctx.enter_context(tc.tile_pool(name="pn", bufs=2))
        ptpool = ctx.enter_context(tc.tile_pool(name="pt", bufs=1))
        avpool = ctx.enter_context(tc.tile_pool(name="av", bufs=2))
        stats = ctx.enter_context(tc.tile_pool(name="stats", bufs=2))
        psum_big = ctx.enter_context(
            tc.tile_pool(name="psum_big", bufs=1, space="PSUM")
        )
        psum_pt = ctx.enter_context(
            tc.tile_pool(name="psum_pt", bufs=1, space="PSUM")
        )
        psum_av = ctx.enter_context(
            tc.tile_pool(name="psum_av", bufs=2, space="PSUM")
        )

        for h in range(HPC):
            # ---- load + transpose Q,K; load V ----
            q_st = stage.tile([128, NB * 64], F32, tag="qk_stage")
            nc.sync.dma_start(
                q_st[:].rearrange("p (c d) -> p c d", d=64),
                q_d[h].rearrange("(c p) d -> p c d", p=128),
            )
            k_st = stage.tile([128, NB * 64], F32, tag="qk_stage")
            nc.sync.dma_start(
                k_st[:].rearrange("p (c d) -> p c d", d=64),
                k_d[h].rearrange("(c p) d -> p c d", p=128),
            )
            v_sb = vpool.tile([128, NB * 64], F32)
            nc.sync.dma_start(
                v_sb[:].rearrange("p (c d) -> p c d", d=64),
                v_d[h].rearrange("(c p) d -> p c d", p=128),
            )

            qt = qtp.tile([64, S], F32)
            kt = ktp.tile([64, S], F32)
            for src, dst, scale in ((q_st, qt, HALF_C), (k_st, kt, None)):
                pt_ps = psum_big.tile([64, S], F32, tag="big")
                for c in range(NB):
                    nc.tensor.matmul(
                        pt_ps[:, ts(c, 128)],
                        src[:, ts(c, 64)],
                        ident[:],
                        is_transpose=True,
                    )
                if scale is None:
                    nc.vector.tensor_copy(dst[:], pt_ps[:])
                else:
                    nc.vector.tensor_scalar_mul(dst[:], pt_ps[:], float(scale))

            # ---- q-block loop, in convoy groups ----
            for g in range(n_blocks // GROUP):
                mv = stats.tile([128, 2 * GROUP], F32, tag="mv")
                se_t = stats.tile([128, GROUP], F32, tag="se")
                rse_t = stats.tile([128, GROUP], F32, tag="rse")
                negD_t = stats.tile([128, GROUP], F32, tag="negD")
                S1_t = stats.tile([128, GROUP], F32, tag="S1")
                s_t = stats.tile([128, GROUP], F32, tag="s")
                e_t = stats.tile([128, GROUP], F32, tag="e")
                r_t = stats.tile([128, GROUP], F32, tag="r")
                d_t = stats.tile([128, GROUP], F32, tag="d")
                negd_t = stats.tile([128, GROUP], F32, tag="negd")
                sig_t = stats.tile([128, GROUP], F32, tag="sig")
                rs_t = stats.tile([128, GROUP], F32, tag="rs")
                t_tiles = []
                psum_list = []

                # phase A: scores -> stats + teacher-exp (psum freed by adv0)
                for bb in range(GROUP):
                    b = g * GROUP + bb
                    y_ps = psum_big.tile([128, S], F32, tag="big")
                    for j in range(4):
                        nc.tensor.matmul(
                            y_ps[:, ts(j, 512)],
                            qt[:, ts(b, 128)],
                            kt[:, ts(j, 512)],
                        )
                    bn6 = stats.tile([128, 24], F32, tag="bn6")
                    for gg in range(4):
                        nc.vector.bn_stats(
                            bn6[:, 6 * gg : 6 * gg + 6],
                            y_ps[:, ts(gg, 512)],
                        )
                    nc.vector.bn_aggr(mv[:, 2 * bb : 2 * bb + 2], bn6[:])
                    # anchor (per-block; keeps PSUM evacuation un-batched):
                    # negD = -(mu + T*(A*var + B))
                    nc.vector.tensor_scalar(
                        sig_t[:, bb : bb + 1], mv[:, 2 * bb + 1 : 2 * bb + 2],
                        float(SIG_A), float(SIG_B), ALU.mult, ALU.add,
                    )
                    nc.vector.scalar_tensor_tensor(
                        negD_t[:, bb : bb + 1], sig_t[:, bb : bb + 1],
                        -float(T_INIT), mv[:, 2 * bb : 2 * bb + 1], ALU.mult,
                        ALU.subtract,
                    )
                    tt = tpool.tile([128, S], F32)
                    nc.scalar.activation(
                        tt[:], y_ps[:], AF.Exp, scale=2.0,
                        accum_out=se_t[:, bb : bb + 1],
                    )
                    t_tiles.append(tt)
                    psum_list.append(y_ps)

                # teacher normalize (per-block recip to avoid pool cycles)
                for bb in range(GROUP):
                    b = g * GROUP + bb
                    nc.vector.reciprocal(
                        rse_t[:, bb : bb + 1], se_t[:, bb : bb + 1]
                    )
                    tn = tnpool.tile([128, S], F32)
                    nc.gpsimd.tensor_scalar(
                        tn[:], t_tiles[bb][:], rse_t[:, bb : bb + 1], None,
                        ALU.mult,
                    )
                    nc.sync.dma_start(te_d[h, ts(b, 128), :], tn[:])

                # Newton iterations: chained u, advance on ACT (Relu+accum S1),
                # measure on DVE (stt, accum s)
                u_prev = [None] * GROUP
                p_tiles = [None] * GROUP
                for it in range(n_meas):
                    last = it == n_meas - 1
                    for bb in range(GROUP):
                        scol = s_t[:, bb : bb + 1]
                        u = upool.tile([128, S], F32, tag="u")
                        if it == 0:
                            nc.scalar.activation(
                                u[:], psum_list[bb][:], AF.Relu,
                                bias=negD_t[:, bb : bb + 1],
                                accum_out=S1_t[:, bb : bb + 1],
                            )
                            w = wpool.tile([128, S], F32, tag="w")
                            nc.vector.scalar_tensor_tensor(
                                w[:], u[:], 0.0, u[:], ALU.subtract, ALU.mult,
                                accum_out=scol,
                            )
                        else:
                            nc.scalar.activation(
                                u[:], u_prev[bb][:], AF.Relu,
                                bias=negd_t[:, bb : bb + 1],
                                accum_out=S1_t[:, bb : bb + 1],
                            )
                            if last:
                                out_t = ppool.tile([128, S], F32, tag="p")
                                p_tiles[bb] = out_t
                            else:
                                out_t = u_prev[bb]
                            nc.vector.scalar_tensor_tensor(
                                out_t[:], u_prev[bb][:], d_t[:, bb : bb + 1],
                                u[:], ALU.subtract, ALU.mult, accum_out=scol,
                            )
                        u_prev[bb] = u
                    if last:
                        break
                    # batched Newton update: d = max(0.5*(s-1)/S1, 0)
                    nc.vector.tensor_scalar_sub(e_t[:], s_t[:], 1.0)
                    nc.vector.tensor_scalar_max(S1_t[:], S1_t[:], 1e-12)
                    nc.vector.reciprocal(r_t[:], S1_t[:])
                    nc.vector.scalar_tensor_tensor(
                        d_t[:], e_t[:], 0.5, r_t[:], ALU.mult, ALU.mult
                    )
                    nc.vector.tensor_scalar_max(d_t[:], d_t[:], 0.0)
                    nc.vector.tensor_scalar_mul(negd_t[:], d_t[:], -1.0)

                # student normalize + transposes + AV + DMA
                for bb in range(GROUP):
                    b = g * GROUP + bb
                    nc.vector.reciprocal(
                        rs_t[:, bb : bb + 1], s_t[:, bb : bb + 1]
                    )
                    p = p_tiles[bb]
                    pn = pnpool.tile([128, S], F32)
                    nc.gpsimd.tensor_scalar(
                        pn[:], p[:], rs_t[:, bb : bb + 1], None, ALU.mult
                    )
                    nc.sync.dma_start(st_d[h, ts(b, 128), :], pn[:])
                    # transpose p_norm 128x128 chunks -> PSUM -> SBUF
                    pt_sb = ptpool.tile([128, S], F32)
                    for half in range(2):
                        pt_ps = psum_pt.tile([128, 1024], F32, tag="pt")
                        for cc in range(8):
                            c = half * 8 + cc
                            nc.tensor.matmul(
                                pt_ps[:, ts(cc, 128)],
                                pn[:, ts(c, 128)],
                                ident[:],
                                is_transpose=True,
                            )
                        nc.scalar.copy(
                            pt_sb[:, ts(half, 1024)], pt_ps[:]
                        )
                    av_ps = psum_av.tile([128, 64], F32, tag="av")
                    for c in range(NB):
                        nc.tensor.matmul(
                            av_ps[:],
                            pt_sb[:, ts(c, 128)],
                            v_sb[:, ts(c, 64)],
                            start=(c == 0),
                            stop=(c == NB - 1),
                        )
                    av_sb = avpool.tile([128, 64], F32)
                    nc.vector.tensor_copy(av_sb[:], av_ps[:])
                    nc.sync.dma_start(o_d[h, ts(b, 128), :], av_sb[:])
    nc.compile()
    return nc


def kernel(query, key, value):
    query = np.ascontiguousarray(query, dtype=np.float32)
    key = np.ascontiguousarray(key, dtype=np.float32)
    value = np.ascontiguousarray(value, dtype=np.float32)

    if "nc" not in _CACHE:
        _CACHE["nc"] = _build_program()
    nc = _CACHE["nc"]

    in_maps = []
    for i in range(NCORES):
        sl = slice(i * HPC, (i + 1) * HPC)
        in_maps.append(
            {
                "q": np.ascontiguousarray(query[0, sl]),
                "k": np.ascontiguousarray(key[0, sl]),
                "v": np.ascontiguousarray(value[0, sl]),
            }
        )
    res = run_bass_kernel_spmd(
        nc, in_maps, core_ids=list(range(NCORES)),
        trace=bool(int(os.environ.get("KBENCH_TRACE", "0"))),
    )
    _CACHE["last_results"] = res

    out = np.empty((B, H, S, D), np.float32)
    student = np.empty((B, H, S, S), np.float32)
    teacher = np.empty((B, H, S, S), np.float32)
    for i in range(NCORES):
        sl = slice(i * HPC, (i + 1) * HPC)
        out[0, sl] = res.results[i]["o"]
        student[0, sl] = res.results[i]["st"]
        teacher[0, sl] = res.results[i]["te"]
    return out, student, teacher
